# revision 14
# baseline (speedup 1.0000x reference)
"""Trainium2 Bass kernel for the EnsembleDynamicsNetwork problem.

Strategy:
- Ensemble-sharding: member e -> NeuronCore e (8 members, 8 cores). Every core
  sees the full batch; members are fully independent (no collectives).
- Input normalization is folded into layer-1 weights/bias on the host;
  output denormalization of the means is folded into the output head weights.
  denorm_stds = exp(clip(ls, -10, .5)) * dstd = exp(clip(ls, -10, .5) + log(dstd)),
  computed with the ACT engine's fused exp(x + bias).
- On-chip activations live transposed: A_l = h_l.T stored as 4x [128, 512chunk]
  SBUF tiles, so every layer is out[c] = sum_k W[kblk, cblk].T-free matmuls with
  plain weight blocks as the stationary operand and NO inter-layer transposes.
- Matmuls run in float32r (full-rate fp32 mode, ~1e-4 rel err), fp32 PSUM.
- Final [feat, batch] -> [batch, feat] layout fix via DVE 32x32 block transpose
  + strided DMA. rewards/dones stay transposed ([2, B]); host adds their scalar
  bias and reshapes.
"""
import sys

sys.path.insert(0, "/opt/trn_rl_repo")

import numpy as np

# --- problem constants (hardcoded; kernel.py must be self-contained) ---
E = 8
H = 512
OBS = 32
ACT_DIM = 16
D_IN = OBS + ACT_DIM  # 48
B = 32768
LOG_STD_MIN = -10.0
LOG_STD_MAX = 0.5
N_CORES = 8
CHUNK = 512
N_CHUNKS = B // CHUNK  # 64
KBLK = H // 128  # 4
OUTW = OBS + OBS + 1 + 1  # 66

_cache = {}


def _build_program():
    import concourse.bass as bass
    import concourse.mybir as mybir
    import concourse.tile as tile
    from concourse import bacc

    F32 = mybir.dt.float32
    F32R = mybir.dt.float32r
    AF = mybir.ActivationFunctionType
    ALU = mybir.AluOpType

    nc = bacc.Bacc("TRN2", target_bir_lowering=False, debug=False)

    # xt is zero-padded from 48 to 128 rows: K=128 weight loads use full
    # row-groups, which lets LDWEIGHTS pipeline behind in-flight matmuls
    # (partial row-group loads serialize, ~320ns vs ~232ns per matmul).
    xt = nc.dram_tensor("xt", [128, B], F32R, kind="ExternalInput")
    w1 = nc.dram_tensor("w1", [128, H], F32R, kind="ExternalInput")
    w2 = nc.dram_tensor("w2", [128, KBLK, H], F32R, kind="ExternalInput")
    w3 = nc.dram_tensor("w3", [128, KBLK, H], F32R, kind="ExternalInput")
    w4 = nc.dram_tensor("w4", [128, KBLK, H], F32R, kind="ExternalInput")
    wo = nc.dram_tensor("wo", [128, KBLK, OUTW], F32R, kind="ExternalInput")
    bh = nc.dram_tensor("bh", [128, 16], F32, kind="ExternalInput")
    bo = nc.dram_tensor("bo", [64, 1], F32, kind="ExternalInput")
    lds = nc.dram_tensor("lds", [64, 1], F32, kind="ExternalInput")

    om = nc.dram_tensor("om", [B, OBS], F32, kind="ExternalOutput")
    os_ = nc.dram_tensor("os", [B, OBS], F32, kind="ExternalOutput")
    ord_ = nc.dram_tensor("ord", [2, B], F32, kind="ExternalOutput")

    with tile.TileContext(nc) as tc:
        with (
            tc.tile_pool(name="wp", bufs=1) as wp,
            tc.tile_pool(name="a0p", bufs=6) as a0p,
            tc.tile_pool(name="ap", bufs=20) as ap,
            tc.tile_pool(name="op", bufs=3) as op,
            tc.tile_pool(name="php", bufs=5, space="PSUM") as php,
            tc.tile_pool(name="pop", bufs=2, space="PSUM") as pop,
        ):
            w1_sb = wp.tile([128, H], F32R, tag="w1")
            w2_sb = wp.tile([128, KBLK, H], F32R, tag="w2")
            w3_sb = wp.tile([128, KBLK, H], F32R, tag="w3")
            w4_sb = wp.tile([128, KBLK, H], F32R, tag="w4")
            wo_sb = wp.tile([128, KBLK, OUTW], F32R, tag="wo")
            bh_sb = wp.tile([128, 16], F32, tag="bh")
            bo_sb = wp.tile([64, 1], F32, tag="bo")
            lds_sb = wp.tile([64, 1], F32, tag="lds")
            nc.sync.dma_start(w1_sb[:], w1[:])
            nc.sync.dma_start(w2_sb[:], w2[:])
            nc.sync.dma_start(w3_sb[:], w3[:])
            nc.sync.dma_start(w4_sb[:], w4[:])
            nc.sync.dma_start(wo_sb[:], wo[:])
            nc.sync.dma_start(bh_sb[:], bh[:])
            nc.sync.dma_start(bo_sb[:], bo[:])
            nc.sync.dma_start(lds_sb[:], lds[:])

            w_mid = [w2_sb, w3_sb, w4_sb]

            def relu_into(dst, src, bias_ap, use_act):
                """dst = relu(src + bias), on ACT or DVE."""
                if use_act:
                    nc.scalar.activation(dst, src, AF.Relu, bias=bias_ap)
                else:
                    nc.vector.tensor_scalar(
                        out=dst, in0=src, scalar1=bias_ap, scalar2=0.0,
                        op0=ALU.add, op1=ALU.max,
                    )

            def load_a0(i):
                a0 = a0p.tile([128, CHUNK], F32R, tag="a0", name=f"a0_{i}")
                nc.sync.dma_start(a0[:], xt[:, bass.ts(i, CHUNK)])
                return a0

            def layer1(i, a0):
                a_out = []
                for c in range(KBLK):
                    ph = php.tile([128, CHUNK], F32, tag="ph", name=f"ph1_{i}_{c}")
                    nc.tensor.matmul(
                        ph[:], w1_sb[:, bass.ts(c, 128)], a0[:],
                        start=True, stop=True,
                    )
                    a = ap.tile([128, CHUNK], F32R, tag="a", name=f"a1_{i}_{c}")
                    relu_into(a[:], ph[:], bh_sb[:, c : c + 1], use_act=(c < 2))
                    a_out.append(a)
                return a_out

            def layer_mid(i, li, a_in):
                w_sb = w_mid[li]
                a_out = []
                for c in range(KBLK):
                    ph = php.tile([128, CHUNK], F32, tag="ph", name=f"ph_{i}_{li}_{c}")
                    for k in range(KBLK):
                        nc.tensor.matmul(
                            ph[:], w_sb[:, k, bass.ts(c, 128)], a_in[k][:],
                            start=(k == 0), stop=(k == KBLK - 1),
                        )
                    a = ap.tile([128, CHUNK], F32R, tag="a", name=f"a_{i}_{li}_{c}")
                    bias_ap = bh_sb[:, 4 * (li + 1) + c : 4 * (li + 1) + c + 1]
                    relu_into(a[:], ph[:], bias_ap, use_act=(c < 2))
                    a_out.append(a)
                return a_out

            def out_head(i, a_in):
                po = pop.tile([OUTW, CHUNK], F32, tag="po", name=f"po_{i}")
                for k in range(KBLK):
                    nc.tensor.matmul(
                        po[:], wo_sb[:, k, :], a_in[k][:],
                        start=(k == 0), stop=(k == KBLK - 1),
                    )
                return po

            def postprocess(i, po):
                cs = bass.ts(i, CHUNK)
                t_out = op.tile([64, CHUNK], F32, tag="t_out", name=f"to_{i}")
                t_cl = op.tile([64, CHUNK], F32, tag="t_cl", name=f"tc_{i}")
                t_mn = op.tile([64, CHUNK], F32, tag="t_mn", name=f"tm_{i}")
                # means: + bias (denorm already folded into weights) — on ACT
                nc.scalar.activation(
                    t_out[0:32, :], po[0:32, :], AF.Identity,
                    bias=bo_sb[0:32, :],
                )
                # logstds: + bias, clip to [-10, 0.5], then exp(x + log(dstd))
                nc.vector.tensor_scalar(
                    out=t_cl[32:64, :], in0=po[32:64, :],
                    scalar1=bo_sb[32:64, :], scalar2=LOG_STD_MIN,
                    op0=ALU.add, op1=ALU.max,
                )
                nc.vector.tensor_scalar_min(
                    out=t_mn[32:64, :], in0=t_cl[32:64, :], scalar1=LOG_STD_MAX,
                )
                nc.scalar.activation(
                    t_out[32:64, :], t_mn[32:64, :], AF.Exp,
                    bias=lds_sb[32:64, :],
                )
                # block-transpose [feat, batch] -> batch-major and store
                t_tr = op.tile([64, CHUNK], F32, tag="t_tr", name=f"tt_{i}")
                nc.vector.transpose(t_tr[:], t_out[:])
                nc.sync.dma_start(
                    om[cs, :].rearrange("(j p) q -> p j q", p=32),
                    t_tr[0:32, :].rearrange("p (j q) -> p j q", q=32),
                )
                nc.sync.dma_start(
                    os_[cs, :].rearrange("(j p) q -> p j q", p=32),
                    t_tr[32:64, :].rearrange("p (j q) -> p j q", q=32),
                )
                # rewards/dones raw (bias added on host), stay transposed.
                # DMA cannot read PSUM -> bounce through SBUF at matching
                # partition offset (engine lanes are partition-aligned).
                t_rd = op.tile([66, CHUNK], F32, tag="t_rd", name=f"tr_{i}")
                nc.scalar.copy(t_rd[64:66, :], po[64:66, :])
                nc.sync.dma_start(ord_[:, cs], t_rd[64:66, :])

            # Chunk PAIRS, layer-interleaved: layer l of chunk B issues between
            # layer l and l+1 of chunk A, so every matmul's A-tile inputs were
            # produced >= one full layer (16 matmuls, ~3.4us) earlier — relu
            # latency never stalls the PE at layer transitions.
            for p in range(N_CHUNKS // 2):
                iA, iB = 2 * p, 2 * p + 1
                a0A, a0B = load_a0(iA), load_a0(iB)
                aA, aB = layer1(iA, a0A), layer1(iB, a0B)
                for li in range(3):
                    aA = layer_mid(iA, li, aA)
                    aB = layer_mid(iB, li, aB)
                poA = out_head(iA, aA)
                poB = out_head(iB, aB)
                postprocess(iA, poA)
                postprocess(iB, poB)

    nc.compile()
    return nc


def _prep_core_inputs(xt_full, W1, b1, W2, b2, W3, b3, W4, b4,
                      Wm, bm, Wls, bls, Wr, br, Wd, bd,
                      state_mean, state_std, action_mean, action_std,
                      delta_mean, delta_std, e):
    f64 = np.float64
    mu = np.concatenate([state_mean, action_mean]).astype(f64)
    sig = np.concatenate([state_std, action_std]).astype(f64)

    W1e = W1[e].astype(f64)
    w1f = W1e / sig[:, None]
    b1f = b1[e].astype(f64) - (mu / sig) @ W1e

    def blocks(w, width):
        return np.ascontiguousarray(
            w.reshape(KBLK, 128, width).transpose(1, 0, 2)
        ).astype(np.float32)

    dstd = delta_std.astype(f64)
    dmean = delta_mean.astype(f64)
    wm_f = Wm[e].astype(f64) * dstd[None, :]
    bo_v = np.concatenate([bm[e].astype(f64) * dstd + dmean, bls[e]]).astype(np.float32)
    wo_full = np.concatenate(
        [wm_f.astype(np.float32), Wls[e], Wr[e], Wd[e]], axis=1
    )  # [H, 66]

    bh_v = np.zeros((128, 16), np.float32)
    for l, bl in enumerate([b1f.astype(np.float32), b2[e], b3[e], b4[e]]):
        for c in range(KBLK):
            bh_v[:, 4 * l + c] = bl[c * 128 : (c + 1) * 128]

    lds_v = np.zeros((64, 1), np.float32)
    lds_v[32:64, 0] = np.log(dstd).astype(np.float32)

    w1p = np.zeros((128, H), np.float32)
    w1p[:D_IN, :] = w1f.astype(np.float32)

    return {
        "xt": xt_full,
        "w1": w1p,
        "w2": blocks(W2[e], H),
        "w3": blocks(W3[e], H),
        "w4": blocks(W4[e], H),
        "wo": blocks(wo_full, OUTW),
        "bh": bh_v,
        "bo": bo_v[:, None].copy(),
        "lds": lds_v,
    }


def kernel(states, actions, state_mean, state_std, action_mean, action_std,
           delta_mean, delta_std,
           W1, b1, W2, b2, W3, b3, W4, b4,
           Wm, bm, Wls, bls, Wr, br, Wd, bd, **run_kwargs):
    from concourse.bass_utils import run_bass_kernel_spmd

    to_np = lambda a: np.asarray(a, dtype=np.float32)
    states, actions = to_np(states), to_np(actions)
    args = [to_np(a) for a in (W1, b1, W2, b2, W3, b3, W4, b4,
                               Wm, bm, Wls, bls, Wr, br, Wd, bd)]
    (W1, b1, W2, b2, W3, b3, W4, b4,
     Wm, bm, Wls, bls, Wr, br, Wd, bd) = args
    norms = [to_np(a) for a in (state_mean, state_std, action_mean, action_std,
                                delta_mean, delta_std)]
    (state_mean, state_std, action_mean, action_std,
     delta_mean, delta_std) = norms

    if "nc" not in _cache:
        _cache["nc"] = _build_program()
    nc = _cache["nc"]

    xt_full = np.zeros((128, B), np.float32)
    xt_full[:D_IN, :] = np.concatenate([states, actions], axis=1).T

    in_maps = [
        _prep_core_inputs(xt_full, W1, b1, W2, b2, W3, b3, W4, b4,
                          Wm, bm, Wls, bls, Wr, br, Wd, bd,
                          state_mean, state_std, action_mean, action_std,
                          delta_mean, delta_std, e)
        for e in range(N_CORES)
    ]

    res = run_bass_kernel_spmd(nc, in_maps, list(range(N_CORES)), **run_kwargs)
    _cache["last_result"] = res

    means = np.stack([res.results[e]["om"] for e in range(N_CORES)])
    stds = np.stack([res.results[e]["os"] for e in range(N_CORES)])
    rd = np.stack([res.results[e]["ord"] for e in range(N_CORES)])  # [E, 2, B]
    rewards = (rd[:, 0, :] + br[:, 0:1]).astype(np.float32)[:, :, None]
    dones = (rd[:, 1, :] + bd[:, 0:1]).astype(np.float32)[:, :, None]
    return means, stds, rewards, dones


# revision 16
# speedup vs baseline: 1.1256x; 1.1256x over previous
"""Trainium2 Bass kernel for the EnsembleDynamicsNetwork problem.

Strategy:
- Ensemble-sharding: member e -> NeuronCore e (8 members, 8 cores). Every core
  sees the full batch; members are fully independent (no collectives).
- Input normalization is folded into layer-1 weights/bias on the host;
  output denormalization of the means is folded into the output head weights.
  denorm_stds = exp(clip(ls, -10, .5)) * dstd = exp(clip(ls, -10, .5) + log(dstd)),
  computed with the ACT engine's fused exp(x + bias).
- On-chip activations live transposed: A_l = h_l.T stored as 4x [128, 512chunk]
  SBUF tiles, so every layer is out[c] = sum_k W[kblk, cblk].T-free matmuls with
  plain weight blocks as the stationary operand and NO inter-layer transposes.
- Matmuls run in float32r (full-rate fp32 mode, ~1e-4 rel err), fp32 PSUM.
- Final [feat, batch] -> [batch, feat] layout fix via DVE 32x32 block transpose
  + strided DMA. rewards/dones stay transposed ([2, B]); host adds their scalar
  bias and reshapes.
"""
import sys

sys.path.insert(0, "/opt/trn_rl_repo")

import numpy as np

# --- problem constants (hardcoded; kernel.py must be self-contained) ---
E = 8
H = 512
OBS = 32
ACT_DIM = 16
D_IN = OBS + ACT_DIM  # 48
B = 32768
LOG_STD_MIN = -10.0
LOG_STD_MAX = 0.5
N_CORES = 8
CHUNK = 512
N_CHUNKS = B // CHUNK  # 64
KBLK = H // 128  # 4
OUTW = OBS + OBS + 1 + 1  # 66

_cache = {}


def _build_program():
    import concourse.bass as bass
    import concourse.mybir as mybir
    import concourse.tile as tile
    from concourse import bacc

    F32 = mybir.dt.float32
    F32R = mybir.dt.float32r
    AF = mybir.ActivationFunctionType
    ALU = mybir.AluOpType

    nc = bacc.Bacc("TRN2", target_bir_lowering=False, debug=False)

    # xt is zero-padded from 48 to 128 rows: K=128 weight loads use full
    # row-groups, which lets LDWEIGHTS pipeline behind in-flight matmuls
    # (partial row-group loads serialize, ~320ns vs ~232ns per matmul).
    xt = nc.dram_tensor("xt", [128, B], F32R, kind="ExternalInput")
    w1 = nc.dram_tensor("w1", [128, H], F32R, kind="ExternalInput")
    w2 = nc.dram_tensor("w2", [128, KBLK, H], F32R, kind="ExternalInput")
    w3 = nc.dram_tensor("w3", [128, KBLK, H], F32R, kind="ExternalInput")
    w4 = nc.dram_tensor("w4", [128, KBLK, H], F32R, kind="ExternalInput")
    wo = nc.dram_tensor("wo", [128, KBLK, OUTW], F32R, kind="ExternalInput")
    bh = nc.dram_tensor("bh", [128, 16], F32, kind="ExternalInput")
    bo = nc.dram_tensor("bo", [64, 1], F32, kind="ExternalInput")
    lds = nc.dram_tensor("lds", [64, 1], F32, kind="ExternalInput")

    om = nc.dram_tensor("om", [B, OBS], F32, kind="ExternalOutput")
    os_ = nc.dram_tensor("os", [B, OBS], F32, kind="ExternalOutput")
    ord_ = nc.dram_tensor("ord", [2, B], F32, kind="ExternalOutput")

    with tile.TileContext(nc) as tc:
        with (
            tc.tile_pool(name="wp", bufs=1) as wp,
            tc.tile_pool(name="a0p", bufs=6) as a0p,
            tc.tile_pool(name="ap", bufs=20) as ap,
            tc.tile_pool(name="op", bufs=3) as op,
            tc.tile_pool(name="php", bufs=4, space="PSUM") as php,
            tc.tile_pool(name="pop", bufs=4, space="PSUM") as pop,
        ):
            w1_sb = wp.tile([128, H], F32R, tag="w1")
            w2_sb = wp.tile([128, KBLK, H], F32R, tag="w2")
            w3_sb = wp.tile([128, KBLK, H], F32R, tag="w3")
            w4_sb = wp.tile([128, KBLK, H], F32R, tag="w4")
            wo_sb = wp.tile([128, KBLK, OUTW], F32R, tag="wo")
            bh_sb = wp.tile([128, 16], F32, tag="bh")
            bo_sb = wp.tile([64, 1], F32, tag="bo")
            lds_sb = wp.tile([64, 1], F32, tag="lds")
            nc.sync.dma_start(w1_sb[:], w1[:])
            nc.sync.dma_start(w2_sb[:], w2[:])
            nc.sync.dma_start(w3_sb[:], w3[:])
            nc.sync.dma_start(w4_sb[:], w4[:])
            nc.sync.dma_start(wo_sb[:], wo[:])
            nc.sync.dma_start(bh_sb[:], bh[:])
            nc.sync.dma_start(bo_sb[:], bo[:])
            nc.sync.dma_start(lds_sb[:], lds[:])

            w_mid = [w2_sb, w3_sb, w4_sb]

            def relu_into(dst, src, bias_ap, use_act):
                """dst = relu(src + bias), on ACT or DVE."""
                if use_act:
                    nc.scalar.activation(dst, src, AF.Relu, bias=bias_ap)
                else:
                    nc.vector.tensor_scalar(
                        out=dst, in0=src, scalar1=bias_ap, scalar2=0.0,
                        op0=ALU.add, op1=ALU.max,
                    )

            def load_a0(i):
                a0 = a0p.tile([128, CHUNK], F32R, tag="a0", name=f"a0_{i}")
                nc.sync.dma_start(a0[:], xt[:, bass.ts(i, CHUNK)])
                return a0

            def layer1(i, a0):
                a_out = []
                for c in range(KBLK):
                    ph = php.tile([128, CHUNK], F32, tag="ph", name=f"ph1_{i}_{c}")
                    nc.tensor.matmul(
                        ph[:], w1_sb[:, bass.ts(c, 128)], a0[:],
                        start=True, stop=True,
                    )
                    a = ap.tile([128, CHUNK], F32R, tag="a", name=f"a1_{i}_{c}")
                    relu_into(a[:], ph[:], bh_sb[:, c : c + 1], use_act=(c < 2))
                    a_out.append(a)
                return a_out

            def layer_mid(i, li, a_in):
                w_sb = w_mid[li]
                a_out = []
                for c in range(KBLK):
                    ph = php.tile([128, CHUNK], F32, tag="ph", name=f"ph_{i}_{li}_{c}")
                    for k in range(KBLK):
                        nc.tensor.matmul(
                            ph[:], w_sb[:, k, bass.ts(c, 128)], a_in[k][:],
                            start=(k == 0), stop=(k == KBLK - 1),
                        )
                    a = ap.tile([128, CHUNK], F32R, tag="a", name=f"a_{i}_{li}_{c}")
                    bias_ap = bh_sb[:, 4 * (li + 1) + c : 4 * (li + 1) + c + 1]
                    relu_into(a[:], ph[:], bias_ap, use_act=(c < 2))
                    a_out.append(a)
                return a_out

            def out_head(i, a_in):
                po = pop.tile([OUTW, CHUNK], F32, tag="po", name=f"po_{i}")
                for k in range(KBLK):
                    nc.tensor.matmul(
                        po[:], wo_sb[:, k, :], a_in[k][:],
                        start=(k == 0), stop=(k == KBLK - 1),
                    )
                return po

            def postprocess(i, po):
                cs = bass.ts(i, CHUNK)
                t_out = op.tile([64, CHUNK], F32, tag="t_out", name=f"to_{i}")
                t_cl = op.tile([64, CHUNK], F32, tag="t_cl", name=f"tc_{i}")
                t_mn = op.tile([64, CHUNK], F32, tag="t_mn", name=f"tm_{i}")
                # means: + bias (denorm already folded into weights) — on ACT
                nc.scalar.activation(
                    t_out[0:32, :], po[0:32, :], AF.Identity,
                    bias=bo_sb[0:32, :],
                )
                # logstds: + bias, clip to [-10, 0.5], then exp(x + log(dstd))
                nc.vector.tensor_scalar(
                    out=t_cl[32:64, :], in0=po[32:64, :],
                    scalar1=bo_sb[32:64, :], scalar2=LOG_STD_MIN,
                    op0=ALU.add, op1=ALU.max,
                )
                nc.vector.tensor_scalar_min(
                    out=t_mn[32:64, :], in0=t_cl[32:64, :], scalar1=LOG_STD_MAX,
                )
                nc.scalar.activation(
                    t_out[32:64, :], t_mn[32:64, :], AF.Exp,
                    bias=lds_sb[32:64, :],
                )
                # block-transpose [feat, batch] -> batch-major and store
                t_tr = op.tile([64, CHUNK], F32, tag="t_tr", name=f"tt_{i}")
                nc.vector.transpose(t_tr[:], t_out[:])
                nc.sync.dma_start(
                    om[cs, :].rearrange("(j p) q -> p j q", p=32),
                    t_tr[0:32, :].rearrange("p (j q) -> p j q", q=32),
                )
                nc.sync.dma_start(
                    os_[cs, :].rearrange("(j p) q -> p j q", p=32),
                    t_tr[32:64, :].rearrange("p (j q) -> p j q", q=32),
                )
                # rewards/dones raw (bias added on host), stay transposed.
                # DMA cannot read PSUM -> bounce through SBUF at matching
                # partition offset (engine lanes are partition-aligned).
                t_rd = op.tile([66, CHUNK], F32, tag="t_rd", name=f"tr_{i}")
                nc.scalar.copy(t_rd[64:66, :], po[64:66, :])
                nc.sync.dma_start(ord_[:, cs], t_rd[64:66, :])

            # Chunk PAIRS, layer-interleaved: layer l of chunk B issues between
            # layer l and l+1 of chunk A, so every matmul's A-tile inputs were
            # produced >= one full layer (16 matmuls, ~3.4us) earlier — relu
            # latency never stalls the PE at layer transitions.
            #
            # Post-processing of pair p is DEFERRED until after pair p+1's
            # matmul/relu stream: each engine has one counting semaphore, so a
            # slow post chain queued between relu streams would head-of-line
            # block the semaphore ticks the next pair's matmuls wait on. By
            # post time all inputs are a full pair (~13us) old, so the post
            # ops stream through ACT/DVE without wait-stalls.
            pending = None
            for p in range(N_CHUNKS // 2):
                iA, iB = 2 * p, 2 * p + 1
                a0A, a0B = load_a0(iA), load_a0(iB)
                aA, aB = layer1(iA, a0A), layer1(iB, a0B)
                for li in range(3):
                    aA = layer_mid(iA, li, aA)
                    aB = layer_mid(iB, li, aB)
                poA = out_head(iA, aA)
                poB = out_head(iB, aB)
                if pending is not None:
                    postprocess(*pending[0])
                    postprocess(*pending[1])
                pending = ((iA, poA), (iB, poB))
            postprocess(*pending[0])
            postprocess(*pending[1])

    nc.compile()
    return nc


def _prep_core_inputs(xt_full, W1, b1, W2, b2, W3, b3, W4, b4,
                      Wm, bm, Wls, bls, Wr, br, Wd, bd,
                      state_mean, state_std, action_mean, action_std,
                      delta_mean, delta_std, e):
    f64 = np.float64
    mu = np.concatenate([state_mean, action_mean]).astype(f64)
    sig = np.concatenate([state_std, action_std]).astype(f64)

    W1e = W1[e].astype(f64)
    w1f = W1e / sig[:, None]
    b1f = b1[e].astype(f64) - (mu / sig) @ W1e

    def blocks(w, width):
        return np.ascontiguousarray(
            w.reshape(KBLK, 128, width).transpose(1, 0, 2)
        ).astype(np.float32)

    dstd = delta_std.astype(f64)
    dmean = delta_mean.astype(f64)
    wm_f = Wm[e].astype(f64) * dstd[None, :]
    bo_v = np.concatenate([bm[e].astype(f64) * dstd + dmean, bls[e]]).astype(np.float32)
    wo_full = np.concatenate(
        [wm_f.astype(np.float32), Wls[e], Wr[e], Wd[e]], axis=1
    )  # [H, 66]

    bh_v = np.zeros((128, 16), np.float32)
    for l, bl in enumerate([b1f.astype(np.float32), b2[e], b3[e], b4[e]]):
        for c in range(KBLK):
            bh_v[:, 4 * l + c] = bl[c * 128 : (c + 1) * 128]

    lds_v = np.zeros((64, 1), np.float32)
    lds_v[32:64, 0] = np.log(dstd).astype(np.float32)

    w1p = np.zeros((128, H), np.float32)
    w1p[:D_IN, :] = w1f.astype(np.float32)

    return {
        "xt": xt_full,
        "w1": w1p,
        "w2": blocks(W2[e], H),
        "w3": blocks(W3[e], H),
        "w4": blocks(W4[e], H),
        "wo": blocks(wo_full, OUTW),
        "bh": bh_v,
        "bo": bo_v[:, None].copy(),
        "lds": lds_v,
    }


def kernel(states, actions, state_mean, state_std, action_mean, action_std,
           delta_mean, delta_std,
           W1, b1, W2, b2, W3, b3, W4, b4,
           Wm, bm, Wls, bls, Wr, br, Wd, bd, **run_kwargs):
    from concourse.bass_utils import run_bass_kernel_spmd

    to_np = lambda a: np.asarray(a, dtype=np.float32)
    states, actions = to_np(states), to_np(actions)
    args = [to_np(a) for a in (W1, b1, W2, b2, W3, b3, W4, b4,
                               Wm, bm, Wls, bls, Wr, br, Wd, bd)]
    (W1, b1, W2, b2, W3, b3, W4, b4,
     Wm, bm, Wls, bls, Wr, br, Wd, bd) = args
    norms = [to_np(a) for a in (state_mean, state_std, action_mean, action_std,
                                delta_mean, delta_std)]
    (state_mean, state_std, action_mean, action_std,
     delta_mean, delta_std) = norms

    if "nc" not in _cache:
        _cache["nc"] = _build_program()
    nc = _cache["nc"]

    xt_full = np.zeros((128, B), np.float32)
    xt_full[:D_IN, :] = np.concatenate([states, actions], axis=1).T

    in_maps = [
        _prep_core_inputs(xt_full, W1, b1, W2, b2, W3, b3, W4, b4,
                          Wm, bm, Wls, bls, Wr, br, Wd, bd,
                          state_mean, state_std, action_mean, action_std,
                          delta_mean, delta_std, e)
        for e in range(N_CORES)
    ]

    res = run_bass_kernel_spmd(nc, in_maps, list(range(N_CORES)), **run_kwargs)
    _cache["last_result"] = res

    means = np.stack([res.results[e]["om"] for e in range(N_CORES)])
    stds = np.stack([res.results[e]["os"] for e in range(N_CORES)])
    rd = np.stack([res.results[e]["ord"] for e in range(N_CORES)])  # [E, 2, B]
    rewards = (rd[:, 0, :] + br[:, 0:1]).astype(np.float32)[:, :, None]
    dones = (rd[:, 1, :] + bd[:, 0:1]).astype(np.float32)[:, :, None]
    return means, stds, rewards, dones


# revision 21
# speedup vs baseline: 1.1269x; 1.0011x over previous
"""Trainium2 Bass kernel for the EnsembleDynamicsNetwork problem.

Strategy:
- Ensemble-sharding: member e -> NeuronCore e (8 members, 8 cores). Every core
  sees the full batch; members are fully independent (no collectives).
- Input normalization is folded into layer-1 weights/bias on the host;
  output denormalization of the means is folded into the output head weights.
  denorm_stds = exp(clip(ls, -10, .5)) * dstd = exp(clip(ls, -10, .5) + log(dstd)),
  computed with the ACT engine's fused exp(x + bias).
- On-chip activations live transposed: A_l = h_l.T stored as 4x [128, 512chunk]
  SBUF tiles, so every layer is out[c] = sum_k W[kblk, cblk].T-free matmuls with
  plain weight blocks as the stationary operand and NO inter-layer transposes.
- Matmuls run in float32r (full-rate fp32 mode, ~1e-4 rel err), fp32 PSUM.
- Final [feat, batch] -> [batch, feat] layout fix via DVE 32x32 block transpose
  + strided DMA. rewards/dones stay transposed ([2, B]); host adds their scalar
  bias and reshapes.
"""
import sys

sys.path.insert(0, "/opt/trn_rl_repo")

import numpy as np

# --- problem constants (hardcoded; kernel.py must be self-contained) ---
E = 8
H = 512
OBS = 32
ACT_DIM = 16
D_IN = OBS + ACT_DIM  # 48
B = 32768
LOG_STD_MIN = -10.0
LOG_STD_MAX = 0.5
N_CORES = 8
CHUNK = 512
N_CHUNKS = B // CHUNK  # 64
KBLK = H // 128  # 4
OUTW = OBS + OBS + 1 + 1  # 66

_cache = {}


def _build_program():
    import concourse.bass as bass
    import concourse.mybir as mybir
    import concourse.tile as tile
    from concourse import bacc

    F32 = mybir.dt.float32
    F32R = mybir.dt.float32r
    AF = mybir.ActivationFunctionType
    ALU = mybir.AluOpType

    nc = bacc.Bacc("TRN2", target_bir_lowering=False, debug=False)

    # xt is zero-padded from 48 to 128 rows: K=128 weight loads use full
    # row-groups, which lets LDWEIGHTS pipeline behind in-flight matmuls
    # (partial row-group loads serialize, ~320ns vs ~232ns per matmul).
    xt = nc.dram_tensor("xt", [128, B], F32R, kind="ExternalInput")
    w1 = nc.dram_tensor("w1", [128, H], F32R, kind="ExternalInput")
    w2 = nc.dram_tensor("w2", [128, KBLK, H], F32R, kind="ExternalInput")
    w3 = nc.dram_tensor("w3", [128, KBLK, H], F32R, kind="ExternalInput")
    w4 = nc.dram_tensor("w4", [128, KBLK, H], F32R, kind="ExternalInput")
    wo = nc.dram_tensor("wo", [128, KBLK, OUTW], F32R, kind="ExternalInput")
    bh = nc.dram_tensor("bh", [128, 16], F32, kind="ExternalInput")
    bo = nc.dram_tensor("bo", [64, 1], F32, kind="ExternalInput")
    lds = nc.dram_tensor("lds", [64, 2], F32, kind="ExternalInput")

    om = nc.dram_tensor("om", [B, OBS], F32, kind="ExternalOutput")
    os_ = nc.dram_tensor("os", [B, OBS], F32, kind="ExternalOutput")
    ord_ = nc.dram_tensor("ord", [2, B], F32, kind="ExternalOutput")

    with tile.TileContext(nc) as tc:
        with (
            tc.tile_pool(name="wp", bufs=1) as wp,
            tc.tile_pool(name="a0p", bufs=6) as a0p,
            tc.tile_pool(name="ap", bufs=20) as ap,
            tc.tile_pool(name="op", bufs=3) as op,
            tc.tile_pool(name="php", bufs=4, space="PSUM") as php,
            tc.tile_pool(name="pop", bufs=4, space="PSUM") as pop,
        ):
            w1_sb = wp.tile([128, H], F32R, tag="w1")
            w2_sb = wp.tile([128, KBLK, H], F32R, tag="w2")
            w3_sb = wp.tile([128, KBLK, H], F32R, tag="w3")
            w4_sb = wp.tile([128, KBLK, H], F32R, tag="w4")
            wo_sb = wp.tile([128, KBLK, OUTW], F32R, tag="wo")
            bh_sb = wp.tile([128, 16], F32, tag="bh")
            bo_sb = wp.tile([64, 1], F32, tag="bo")
            lds_sb = wp.tile([64, 2], F32, tag="lds")
            nc.sync.dma_start(w1_sb[:], w1[:])
            nc.sync.dma_start(w2_sb[:], w2[:])
            nc.sync.dma_start(w3_sb[:], w3[:])
            nc.sync.dma_start(w4_sb[:], w4[:])
            nc.sync.dma_start(wo_sb[:], wo[:])
            nc.sync.dma_start(bh_sb[:], bh[:])
            nc.sync.dma_start(bo_sb[:], bo[:])
            nc.sync.dma_start(lds_sb[:], lds[:])

            w_mid = [w2_sb, w3_sb, w4_sb]

            def relu_into(dst, src, bias_ap, use_act):
                """dst = relu(src + bias), on ACT or DVE."""
                if use_act:
                    nc.scalar.activation(dst, src, AF.Relu, bias=bias_ap)
                else:
                    nc.vector.tensor_scalar(
                        out=dst, in0=src, scalar1=bias_ap, scalar2=0.0,
                        op0=ALU.add, op1=ALU.max,
                    )

            def load_a0(i):
                a0 = a0p.tile([128, CHUNK], F32R, tag="a0", name=f"a0_{i}")
                nc.sync.dma_start(a0[:], xt[:, bass.ts(i, CHUNK)])
                return a0

            def layer1(i, a0):
                a_out = []
                for c in range(KBLK):
                    ph = php.tile([128, CHUNK], F32, tag="ph", name=f"ph1_{i}_{c}")
                    nc.tensor.matmul(
                        ph[:], w1_sb[:, bass.ts(c, 128)], a0[:],
                        start=True, stop=True,
                    )
                    a = ap.tile([128, CHUNK], F32R, tag="a", name=f"a1_{i}_{c}")
                    relu_into(a[:], ph[:], bh_sb[:, c : c + 1], use_act=(c < 2))
                    a_out.append(a)
                return a_out

            def layer_mid(i, li, a_in):
                w_sb = w_mid[li]
                a_out = []
                for c in range(KBLK):
                    ph = php.tile([128, CHUNK], F32, tag="ph", name=f"ph_{i}_{li}_{c}")
                    for k in range(KBLK):
                        nc.tensor.matmul(
                            ph[:], w_sb[:, k, bass.ts(c, 128)], a_in[k][:],
                            start=(k == 0), stop=(k == KBLK - 1),
                        )
                    a = ap.tile([128, CHUNK], F32R, tag="a", name=f"a_{i}_{li}_{c}")
                    bias_ap = bh_sb[:, 4 * (li + 1) + c : 4 * (li + 1) + c + 1]
                    relu_into(a[:], ph[:], bias_ap, use_act=(c < 2))
                    a_out.append(a)
                return a_out

            def out_head(i, a_in):
                po = pop.tile([OUTW, CHUNK], F32, tag="po", name=f"po_{i}")
                for k in range(KBLK):
                    nc.tensor.matmul(
                        po[:], wo_sb[:, k, :], a_in[k][:],
                        start=(k == 0), stop=(k == KBLK - 1),
                    )
                return po

            # Post-processing, split into stages that are emitted a full layer
            # (~3.4us) apart so no engine ever dequeues an op whose cross-engine
            # dependency is still pending (single counting sem per engine ->
            # any wait at the queue head stalls everything behind it).
            #
            # The logstd clip+exp runs entirely on ACT via relu identities:
            #   z  = max(x + bls, -10)  = relu(x + (bls+10)) - 10
            #   w  = min(z, 0.5)        = 0.5 - relu(10.5 - relu(x + bls + 10))
            #   std = exp(w + log(dstd)) = exp(-r2 + (0.5 + log(dstd)))
            # bo rows 32:64 hold bls+10; lds rows 32:64 hold 0.5+log(dstd).
            def post_stage1(i, po):
                t_out = op.tile([64, CHUNK], F32, tag="t_out", name=f"to_{i}")
                t_r1 = op.tile([64, CHUNK], F32, tag="t_r1", name=f"t1_{i}")
                # means: + bias (denorm folded into weights) — DVE
                nc.vector.tensor_scalar_add(
                    out=t_out[0:32, :], in0=po[0:32, :], scalar1=bo_sb[0:32, :],
                )
                # r1 = relu(x + bls + 10) — ACT
                nc.scalar.activation(
                    t_r1[32:64, :], po[32:64, :], AF.Relu, bias=bo_sb[32:64, :],
                )
                # rewards/dones raw (bias added on host; DMA can't read PSUM)
                t_rd = op.tile([66, CHUNK], F32, tag="t_rd", name=f"tr_{i}")
                nc.scalar.copy(t_rd[64:66, :], po[64:66, :])
                return t_out, t_r1, t_rd

            def post_stage2(i, st):
                t_out, t_r1, t_rd = st
                t_r2 = op.tile([64, CHUNK], F32, tag="t_r2", name=f"t2_{i}")
                # r2 = relu(10.5 - r1) — ACT
                nc.scalar.activation(
                    t_r2[32:64, :], t_r1[32:64, :], AF.Relu,
                    bias=lds_sb[32:64, 1:2], scale=-1.0,
                )
                return t_out, t_r2, t_rd

            def post_stage3(i, st):
                t_out, t_r2, t_rd = st
                # std = exp(-r2 + (0.5 + log(dstd))) — ACT
                nc.scalar.activation(
                    t_out[32:64, :], t_r2[32:64, :], AF.Exp,
                    bias=lds_sb[32:64, 0:1], scale=-1.0,
                )
                return t_out, t_rd

            def post_stage4(i, st):
                t_out, t_rd = st
                cs = bass.ts(i, CHUNK)
                # block-transpose [feat, batch] -> batch-major and store
                t_tr = op.tile([64, CHUNK], F32, tag="t_tr", name=f"tt_{i}")
                nc.vector.transpose(t_tr[:], t_out[:])
                nc.sync.dma_start(
                    om[cs, :].rearrange("(j p) q -> p j q", p=32),
                    t_tr[0:32, :].rearrange("p (j q) -> p j q", q=32),
                )
                nc.sync.dma_start(
                    os_[cs, :].rearrange("(j p) q -> p j q", p=32),
                    t_tr[32:64, :].rearrange("p (j q) -> p j q", q=32),
                )
                nc.sync.dma_start(ord_[:, cs], t_rd[64:66, :])

            # Chunk PAIRS, layer-interleaved: layer l of chunk B issues between
            # layer l and l+1 of chunk A, so every matmul's A-tile inputs were
            # produced >= one full layer (16 matmuls, ~3.4us) earlier — relu
            # latency never stalls the PE at layer transitions.
            #
            # Post-processing of pair p is DEFERRED until after pair p+1's
            # matmul/relu stream: each engine has one counting semaphore, so a
            # slow post chain queued between relu streams would head-of-line
            # block the semaphore ticks the next pair's matmuls wait on. By
            # post time all inputs are a full pair (~13us) old, so the post
            # ops stream through ACT/DVE without wait-stalls.
            def run_post_all(pend):
                for i, st in pend:
                    st = post_stage2(i, post_stage1(i, st))
                    post_stage4(i, post_stage3(i, st))

            pending = None
            for p in range(N_CHUNKS // 2):
                iA, iB = 2 * p, 2 * p + 1
                a0A, a0B = load_a0(iA), load_a0(iB)
                aA, aB = layer1(iA, a0A), layer1(iB, a0B)
                if pending is not None:
                    st = [post_stage1(i, po) for i, po in pending]
                aA, aB = layer_mid(iA, 0, aA), layer_mid(iB, 0, aB)
                if pending is not None:
                    st = [post_stage2(pending[j][0], s) for j, s in enumerate(st)]
                aA, aB = layer_mid(iA, 1, aA), layer_mid(iB, 1, aB)
                if pending is not None:
                    st = [post_stage3(pending[j][0], s) for j, s in enumerate(st)]
                aA, aB = layer_mid(iA, 2, aA), layer_mid(iB, 2, aB)
                if pending is not None:
                    for j, s in enumerate(st):
                        post_stage4(pending[j][0], s)
                poA = out_head(iA, aA)
                poB = out_head(iB, aB)
                pending = ((iA, poA), (iB, poB))
            run_post_all(pending)

    nc.compile()
    return nc


def _prep_core_inputs(xt_full, W1, b1, W2, b2, W3, b3, W4, b4,
                      Wm, bm, Wls, bls, Wr, br, Wd, bd,
                      state_mean, state_std, action_mean, action_std,
                      delta_mean, delta_std, e):
    f64 = np.float64
    mu = np.concatenate([state_mean, action_mean]).astype(f64)
    sig = np.concatenate([state_std, action_std]).astype(f64)

    W1e = W1[e].astype(f64)
    w1f = W1e / sig[:, None]
    b1f = b1[e].astype(f64) - (mu / sig) @ W1e

    def blocks(w, width):
        return np.ascontiguousarray(
            w.reshape(KBLK, 128, width).transpose(1, 0, 2)
        ).astype(np.float32)

    dstd = delta_std.astype(f64)
    dmean = delta_mean.astype(f64)
    wm_f = Wm[e].astype(f64) * dstd[None, :]
    # rows 0:32: means bias (denorm folded); rows 32:64: bls + 10 (for the
    # relu-based lower clip on ACT)
    bo_v = np.concatenate(
        [bm[e].astype(f64) * dstd + dmean, bls[e].astype(f64) + 10.0]
    ).astype(np.float32)
    wo_full = np.concatenate(
        [wm_f.astype(np.float32), Wls[e], Wr[e], Wd[e]], axis=1
    )  # [H, 66]

    bh_v = np.zeros((128, 16), np.float32)
    for l, bl in enumerate([b1f.astype(np.float32), b2[e], b3[e], b4[e]]):
        for c in range(KBLK):
            bh_v[:, 4 * l + c] = bl[c * 128 : (c + 1) * 128]

    lds_v = np.zeros((64, 2), np.float32)
    lds_v[32:64, 0] = (0.5 + np.log(dstd)).astype(np.float32)
    lds_v[32:64, 1] = 10.5

    w1p = np.zeros((128, H), np.float32)
    w1p[:D_IN, :] = w1f.astype(np.float32)

    return {
        "xt": xt_full,
        "w1": w1p,
        "w2": blocks(W2[e], H),
        "w3": blocks(W3[e], H),
        "w4": blocks(W4[e], H),
        "wo": blocks(wo_full, OUTW),
        "bh": bh_v,
        "bo": bo_v[:, None].copy(),
        "lds": lds_v,
    }


def kernel(states, actions, state_mean, state_std, action_mean, action_std,
           delta_mean, delta_std,
           W1, b1, W2, b2, W3, b3, W4, b4,
           Wm, bm, Wls, bls, Wr, br, Wd, bd, **run_kwargs):
    from concourse.bass_utils import run_bass_kernel_spmd

    to_np = lambda a: np.asarray(a, dtype=np.float32)
    states, actions = to_np(states), to_np(actions)
    args = [to_np(a) for a in (W1, b1, W2, b2, W3, b3, W4, b4,
                               Wm, bm, Wls, bls, Wr, br, Wd, bd)]
    (W1, b1, W2, b2, W3, b3, W4, b4,
     Wm, bm, Wls, bls, Wr, br, Wd, bd) = args
    norms = [to_np(a) for a in (state_mean, state_std, action_mean, action_std,
                                delta_mean, delta_std)]
    (state_mean, state_std, action_mean, action_std,
     delta_mean, delta_std) = norms

    if "nc" not in _cache:
        _cache["nc"] = _build_program()
    nc = _cache["nc"]

    xt_full = np.zeros((128, B), np.float32)
    xt_full[:D_IN, :] = np.concatenate([states, actions], axis=1).T

    in_maps = [
        _prep_core_inputs(xt_full, W1, b1, W2, b2, W3, b3, W4, b4,
                          Wm, bm, Wls, bls, Wr, br, Wd, bd,
                          state_mean, state_std, action_mean, action_std,
                          delta_mean, delta_std, e)
        for e in range(N_CORES)
    ]

    res = run_bass_kernel_spmd(nc, in_maps, list(range(N_CORES)), **run_kwargs)
    _cache["last_result"] = res

    means = np.stack([res.results[e]["om"] for e in range(N_CORES)])
    stds = np.stack([res.results[e]["os"] for e in range(N_CORES)])
    rd = np.stack([res.results[e]["ord"] for e in range(N_CORES)])  # [E, 2, B]
    rewards = (rd[:, 0, :] + br[:, 0:1]).astype(np.float32)[:, :, None]
    dones = (rd[:, 1, :] + bd[:, 0:1]).astype(np.float32)[:, :, None]
    return means, stds, rewards, dones


# revision 22
# speedup vs baseline: 1.1664x; 1.0350x over previous
"""Trainium2 Bass kernel for the EnsembleDynamicsNetwork problem.

Strategy:
- Ensemble-sharding: member e -> NeuronCore e (8 members, 8 cores). Every core
  sees the full batch; members are fully independent (no collectives).
- Input normalization is folded into layer-1 weights/bias on the host;
  output denormalization of the means is folded into the output head weights.
  denorm_stds = exp(clip(ls, -10, .5)) * dstd = exp(clip(ls, -10, .5) + log(dstd)),
  computed with the ACT engine's fused exp(x + bias).
- On-chip activations live transposed: A_l = h_l.T stored as 4x [128, 512chunk]
  SBUF tiles, so every layer is out[c] = sum_k W[kblk, cblk].T-free matmuls with
  plain weight blocks as the stationary operand and NO inter-layer transposes.
- Matmuls run in float32r (full-rate fp32 mode, ~1e-4 rel err), fp32 PSUM.
- Final [feat, batch] -> [batch, feat] layout fix via DVE 32x32 block transpose
  + strided DMA. rewards/dones stay transposed ([2, B]); host adds their scalar
  bias and reshapes.
"""
import sys

sys.path.insert(0, "/opt/trn_rl_repo")

import numpy as np

# --- problem constants (hardcoded; kernel.py must be self-contained) ---
E = 8
H = 512
OBS = 32
ACT_DIM = 16
D_IN = OBS + ACT_DIM  # 48
B = 32768
LOG_STD_MIN = -10.0
LOG_STD_MAX = 0.5
N_CORES = 8
CHUNK = 512
N_CHUNKS = B // CHUNK  # 64
KBLK = H // 128  # 4
OUTW = OBS + OBS + 1 + 1  # 66

_cache = {}


def _build_program():
    import concourse.bass as bass
    import concourse.mybir as mybir
    import concourse.tile as tile
    from concourse import bacc

    F32 = mybir.dt.float32
    F32R = mybir.dt.float32r
    AF = mybir.ActivationFunctionType
    ALU = mybir.AluOpType

    nc = bacc.Bacc("TRN2", target_bir_lowering=False, debug=False)

    # xt is zero-padded from 48 to 128 rows: K=128 weight loads use full
    # row-groups, which lets LDWEIGHTS pipeline behind in-flight matmuls
    # (partial row-group loads serialize, ~320ns vs ~232ns per matmul).
    xt = nc.dram_tensor("xt", [128, B], F32R, kind="ExternalInput")
    w1 = nc.dram_tensor("w1", [128, H], F32R, kind="ExternalInput")
    w2 = nc.dram_tensor("w2", [128, KBLK, H], F32R, kind="ExternalInput")
    w3 = nc.dram_tensor("w3", [128, KBLK, H], F32R, kind="ExternalInput")
    w4 = nc.dram_tensor("w4", [128, KBLK, H], F32R, kind="ExternalInput")
    wo = nc.dram_tensor("wo", [128, KBLK, OUTW], F32R, kind="ExternalInput")
    bh = nc.dram_tensor("bh", [128, 16], F32, kind="ExternalInput")
    bo = nc.dram_tensor("bo", [64, 1], F32, kind="ExternalInput")
    lds = nc.dram_tensor("lds", [64, 2], F32, kind="ExternalInput")

    om = nc.dram_tensor("om", [B, OBS], F32, kind="ExternalOutput")
    os_ = nc.dram_tensor("os", [B, OBS], F32, kind="ExternalOutput")
    ord_ = nc.dram_tensor("ord", [2, B], F32, kind="ExternalOutput")

    with tile.TileContext(nc) as tc:
        with (
            tc.tile_pool(name="wp", bufs=1) as wp,
            tc.tile_pool(name="a0p", bufs=6) as a0p,
            tc.tile_pool(name="ap", bufs=20) as ap,
            tc.tile_pool(name="op", bufs=3) as op,
            tc.tile_pool(name="php", bufs=5, space="PSUM") as php,
            tc.tile_pool(name="pop", bufs=3, space="PSUM") as pop,
        ):
            w1_sb = wp.tile([128, H], F32R, tag="w1")
            w2_sb = wp.tile([128, KBLK, H], F32R, tag="w2")
            w3_sb = wp.tile([128, KBLK, H], F32R, tag="w3")
            w4_sb = wp.tile([128, KBLK, H], F32R, tag="w4")
            wo_sb = wp.tile([128, KBLK, OUTW], F32R, tag="wo")
            bh_sb = wp.tile([128, 16], F32, tag="bh")
            bo_sb = wp.tile([64, 1], F32, tag="bo")
            lds_sb = wp.tile([64, 2], F32, tag="lds")
            nc.sync.dma_start(w1_sb[:], w1[:])
            nc.sync.dma_start(w2_sb[:], w2[:])
            nc.sync.dma_start(w3_sb[:], w3[:])
            nc.sync.dma_start(w4_sb[:], w4[:])
            nc.sync.dma_start(wo_sb[:], wo[:])
            nc.sync.dma_start(bh_sb[:], bh[:])
            nc.sync.dma_start(bo_sb[:], bo[:])
            nc.sync.dma_start(lds_sb[:], lds[:])

            w_mid = [w2_sb, w3_sb, w4_sb]

            def relu_into(dst, src, bias_ap, use_act):
                """dst = relu(src + bias), on ACT or DVE."""
                if use_act:
                    nc.scalar.activation(dst, src, AF.Relu, bias=bias_ap)
                else:
                    nc.vector.tensor_scalar(
                        out=dst, in0=src, scalar1=bias_ap, scalar2=0.0,
                        op0=ALU.add, op1=ALU.max,
                    )

            def load_a0(i):
                a0 = a0p.tile([128, CHUNK], F32R, tag="a0", name=f"a0_{i}")
                nc.sync.dma_start(a0[:], xt[:, bass.ts(i, CHUNK)])
                return a0

            def layer1(i, a0):
                a_out = []
                for c in range(KBLK):
                    ph = php.tile([128, CHUNK], F32, tag="ph", name=f"ph1_{i}_{c}")
                    nc.tensor.matmul(
                        ph[:], w1_sb[:, bass.ts(c, 128)], a0[:],
                        start=True, stop=True,
                    )
                    a = ap.tile([128, CHUNK], F32R, tag="a", name=f"a1_{i}_{c}")
                    relu_into(a[:], ph[:], bh_sb[:, c : c + 1], use_act=(c < 2))
                    a_out.append(a)
                return a_out

            def layer_mid(i, li, a_in):
                w_sb = w_mid[li]
                a_out = []
                for c in range(KBLK):
                    ph = php.tile([128, CHUNK], F32, tag="ph", name=f"ph_{i}_{li}_{c}")
                    for k in range(KBLK):
                        nc.tensor.matmul(
                            ph[:], w_sb[:, k, bass.ts(c, 128)], a_in[k][:],
                            start=(k == 0), stop=(k == KBLK - 1),
                        )
                    a = ap.tile([128, CHUNK], F32R, tag="a", name=f"a_{i}_{li}_{c}")
                    bias_ap = bh_sb[:, 4 * (li + 1) + c : 4 * (li + 1) + c + 1]
                    relu_into(a[:], ph[:], bias_ap, use_act=(c < 2))
                    a_out.append(a)
                return a_out

            def out_head(i, a_in):
                po = pop.tile([OUTW, CHUNK], F32, tag="po", name=f"po_{i}")
                for k in range(KBLK):
                    nc.tensor.matmul(
                        po[:], wo_sb[:, k, :], a_in[k][:],
                        start=(k == 0), stop=(k == KBLK - 1),
                    )
                return po

            # Post-processing, split into stages that are emitted a full layer
            # (~3.4us) apart so no engine ever dequeues an op whose cross-engine
            # dependency is still pending (single counting sem per engine ->
            # any wait at the queue head stalls everything behind it).
            #
            # The logstd clip+exp runs entirely on ACT via relu identities:
            #   z  = max(x + bls, -10)  = relu(x + (bls+10)) - 10
            #   w  = min(z, 0.5)        = 0.5 - relu(10.5 - relu(x + bls + 10))
            #   std = exp(w + log(dstd)) = exp(-r2 + (0.5 + log(dstd)))
            # bo rows 32:64 hold bls+10; lds rows 32:64 hold 0.5+log(dstd).
            def post_stage1(i, po):
                t_out = op.tile([64, CHUNK], F32, tag="t_out", name=f"to_{i}")
                t_r1 = op.tile([64, CHUNK], F32, tag="t_r1", name=f"t1_{i}")
                # means: + bias (denorm folded into weights) — DVE
                nc.vector.tensor_scalar_add(
                    out=t_out[0:32, :], in0=po[0:32, :], scalar1=bo_sb[0:32, :],
                )
                # r1 = relu(x + bls + 10) — ACT
                nc.scalar.activation(
                    t_r1[32:64, :], po[32:64, :], AF.Relu, bias=bo_sb[32:64, :],
                )
                # rewards/dones raw (bias added on host; DMA can't read PSUM)
                t_rd = op.tile([66, CHUNK], F32, tag="t_rd", name=f"tr_{i}")
                nc.scalar.copy(t_rd[64:66, :], po[64:66, :])
                return t_out, t_r1, t_rd

            def post_stage2(i, st):
                t_out, t_r1, t_rd = st
                t_r2 = op.tile([64, CHUNK], F32, tag="t_r2", name=f"t2_{i}")
                # r2 = relu(10.5 - r1) — ACT
                nc.scalar.activation(
                    t_r2[32:64, :], t_r1[32:64, :], AF.Relu,
                    bias=lds_sb[32:64, 1:2], scale=-1.0,
                )
                return t_out, t_r2, t_rd

            def post_stage3(i, st):
                t_out, t_r2, t_rd = st
                # std = exp(-r2 + (0.5 + log(dstd))) — ACT
                nc.scalar.activation(
                    t_out[32:64, :], t_r2[32:64, :], AF.Exp,
                    bias=lds_sb[32:64, 0:1], scale=-1.0,
                )
                return t_out, t_rd

            def post_stage4(i, st):
                t_out, t_rd = st
                cs = bass.ts(i, CHUNK)
                # block-transpose [feat, batch] -> batch-major and store
                t_tr = op.tile([64, CHUNK], F32, tag="t_tr", name=f"tt_{i}")
                nc.vector.transpose(t_tr[:], t_out[:])
                nc.sync.dma_start(
                    om[cs, :].rearrange("(j p) q -> p j q", p=32),
                    t_tr[0:32, :].rearrange("p (j q) -> p j q", q=32),
                )
                nc.sync.dma_start(
                    os_[cs, :].rearrange("(j p) q -> p j q", p=32),
                    t_tr[32:64, :].rearrange("p (j q) -> p j q", q=32),
                )
                nc.sync.dma_start(ord_[:, cs], t_rd[64:66, :])

            # Chunk PAIRS, layer-interleaved: layer l of chunk B issues between
            # layer l and l+1 of chunk A, so every matmul's A-tile inputs were
            # produced >= one full layer (16 matmuls, ~3.4us) earlier — relu
            # latency never stalls the PE at layer transitions.
            #
            # Post-processing of pair p is DEFERRED until after pair p+1's
            # matmul/relu stream: each engine has one counting semaphore, so a
            # slow post chain queued between relu streams would head-of-line
            # block the semaphore ticks the next pair's matmuls wait on. By
            # post time all inputs are a full pair (~13us) old, so the post
            # ops stream through ACT/DVE without wait-stalls.
            def run_post_all(pend):
                for i, st in pend:
                    st = post_stage2(i, post_stage1(i, st))
                    post_stage4(i, post_stage3(i, st))

            pending = None
            for p in range(N_CHUNKS // 2):
                iA, iB = 2 * p, 2 * p + 1
                a0A, a0B = load_a0(iA), load_a0(iB)
                aA, aB = layer1(iA, a0A), layer1(iB, a0B)
                if pending is not None:
                    st = [post_stage1(i, po) for i, po in pending]
                aA, aB = layer_mid(iA, 0, aA), layer_mid(iB, 0, aB)
                if pending is not None:
                    st = [post_stage2(pending[j][0], s) for j, s in enumerate(st)]
                aA, aB = layer_mid(iA, 1, aA), layer_mid(iB, 1, aB)
                if pending is not None:
                    st = [post_stage3(pending[j][0], s) for j, s in enumerate(st)]
                aA, aB = layer_mid(iA, 2, aA), layer_mid(iB, 2, aB)
                if pending is not None:
                    for j, s in enumerate(st):
                        post_stage4(pending[j][0], s)
                poA = out_head(iA, aA)
                poB = out_head(iB, aB)
                pending = ((iA, poA), (iB, poB))
            run_post_all(pending)

    nc.compile()
    return nc


def _prep_core_inputs(xt_full, W1, b1, W2, b2, W3, b3, W4, b4,
                      Wm, bm, Wls, bls, Wr, br, Wd, bd,
                      state_mean, state_std, action_mean, action_std,
                      delta_mean, delta_std, e):
    f64 = np.float64
    mu = np.concatenate([state_mean, action_mean]).astype(f64)
    sig = np.concatenate([state_std, action_std]).astype(f64)

    W1e = W1[e].astype(f64)
    w1f = W1e / sig[:, None]
    b1f = b1[e].astype(f64) - (mu / sig) @ W1e

    def blocks(w, width):
        return np.ascontiguousarray(
            w.reshape(KBLK, 128, width).transpose(1, 0, 2)
        ).astype(np.float32)

    dstd = delta_std.astype(f64)
    dmean = delta_mean.astype(f64)
    wm_f = Wm[e].astype(f64) * dstd[None, :]
    # rows 0:32: means bias (denorm folded); rows 32:64: bls + 10 (for the
    # relu-based lower clip on ACT)
    bo_v = np.concatenate(
        [bm[e].astype(f64) * dstd + dmean, bls[e].astype(f64) + 10.0]
    ).astype(np.float32)
    wo_full = np.concatenate(
        [wm_f.astype(np.float32), Wls[e], Wr[e], Wd[e]], axis=1
    )  # [H, 66]

    bh_v = np.zeros((128, 16), np.float32)
    for l, bl in enumerate([b1f.astype(np.float32), b2[e], b3[e], b4[e]]):
        for c in range(KBLK):
            bh_v[:, 4 * l + c] = bl[c * 128 : (c + 1) * 128]

    lds_v = np.zeros((64, 2), np.float32)
    lds_v[32:64, 0] = (0.5 + np.log(dstd)).astype(np.float32)
    lds_v[32:64, 1] = 10.5

    w1p = np.zeros((128, H), np.float32)
    w1p[:D_IN, :] = w1f.astype(np.float32)

    return {
        "xt": xt_full,
        "w1": w1p,
        "w2": blocks(W2[e], H),
        "w3": blocks(W3[e], H),
        "w4": blocks(W4[e], H),
        "wo": blocks(wo_full, OUTW),
        "bh": bh_v,
        "bo": bo_v[:, None].copy(),
        "lds": lds_v,
    }


def kernel(states, actions, state_mean, state_std, action_mean, action_std,
           delta_mean, delta_std,
           W1, b1, W2, b2, W3, b3, W4, b4,
           Wm, bm, Wls, bls, Wr, br, Wd, bd, **run_kwargs):
    from concourse.bass_utils import run_bass_kernel_spmd

    to_np = lambda a: np.asarray(a, dtype=np.float32)
    states, actions = to_np(states), to_np(actions)
    args = [to_np(a) for a in (W1, b1, W2, b2, W3, b3, W4, b4,
                               Wm, bm, Wls, bls, Wr, br, Wd, bd)]
    (W1, b1, W2, b2, W3, b3, W4, b4,
     Wm, bm, Wls, bls, Wr, br, Wd, bd) = args
    norms = [to_np(a) for a in (state_mean, state_std, action_mean, action_std,
                                delta_mean, delta_std)]
    (state_mean, state_std, action_mean, action_std,
     delta_mean, delta_std) = norms

    if "nc" not in _cache:
        _cache["nc"] = _build_program()
    nc = _cache["nc"]

    xt_full = np.zeros((128, B), np.float32)
    xt_full[:D_IN, :] = np.concatenate([states, actions], axis=1).T

    in_maps = [
        _prep_core_inputs(xt_full, W1, b1, W2, b2, W3, b3, W4, b4,
                          Wm, bm, Wls, bls, Wr, br, Wd, bd,
                          state_mean, state_std, action_mean, action_std,
                          delta_mean, delta_std, e)
        for e in range(N_CORES)
    ]

    res = run_bass_kernel_spmd(nc, in_maps, list(range(N_CORES)), **run_kwargs)
    _cache["last_result"] = res

    means = np.stack([res.results[e]["om"] for e in range(N_CORES)])
    stds = np.stack([res.results[e]["os"] for e in range(N_CORES)])
    rd = np.stack([res.results[e]["ord"] for e in range(N_CORES)])  # [E, 2, B]
    rewards = (rd[:, 0, :] + br[:, 0:1]).astype(np.float32)[:, :, None]
    dones = (rd[:, 1, :] + bd[:, 0:1]).astype(np.float32)[:, :, None]
    return means, stds, rewards, dones


# revision 24
# speedup vs baseline: 1.2548x; 1.0758x over previous
"""Trainium2 Bass kernel for the EnsembleDynamicsNetwork problem.

Strategy:
- Ensemble-sharding: member e -> NeuronCore e (8 members, 8 cores). Every core
  sees the full batch; members are fully independent (no collectives).
- Input normalization is folded into layer-1 weights/bias on the host;
  output denormalization of the means is folded into the output head weights.
  denorm_stds = exp(clip(ls, -10, .5)) * dstd = exp(clip(ls, -10, .5) + log(dstd)),
  computed with the ACT engine's fused exp(x + bias).
- On-chip activations live transposed: A_l = h_l.T stored as 4x [128, 512chunk]
  SBUF tiles, so every layer is out[c] = sum_k W[kblk, cblk].T-free matmuls with
  plain weight blocks as the stationary operand and NO inter-layer transposes.
- Matmuls run in float32r (full-rate fp32 mode, ~1e-4 rel err), fp32 PSUM.
- Final [feat, batch] -> [batch, feat] layout fix via DVE 32x32 block transpose
  + strided DMA. rewards/dones stay transposed ([2, B]); host adds their scalar
  bias and reshapes.
"""
import sys

sys.path.insert(0, "/opt/trn_rl_repo")

import numpy as np

# --- problem constants (hardcoded; kernel.py must be self-contained) ---
E = 8
H = 512
OBS = 32
ACT_DIM = 16
D_IN = OBS + ACT_DIM  # 48
B = 32768
LOG_STD_MIN = -10.0
LOG_STD_MAX = 0.5
N_CORES = 8
CHUNK = 512
N_CHUNKS = B // CHUNK  # 64
KBLK = H // 128  # 4
OUTW = OBS + OBS + 1 + 1  # 66

WEIGHT_BF16 = False

_cache = {}


def _build_program():
    import concourse.bass as bass
    import concourse.mybir as mybir
    import concourse.tile as tile
    from concourse import bacc

    F32 = mybir.dt.float32
    F32R = mybir.dt.float32r
    WDT = mybir.dt.bfloat16 if WEIGHT_BF16 else F32R
    ADT = mybir.dt.bfloat16 if WEIGHT_BF16 else F32R
    AF = mybir.ActivationFunctionType
    ALU = mybir.AluOpType

    nc = bacc.Bacc("TRN2", target_bir_lowering=False, debug=False)

    # xt is zero-padded from 48 to 128 rows: K=128 weight loads use full
    # row-groups, which lets LDWEIGHTS pipeline behind in-flight matmuls
    # (partial row-group loads serialize, ~320ns vs ~232ns per matmul).
    xt = nc.dram_tensor("xt", [128, B], ADT, kind="ExternalInput")
    w1 = nc.dram_tensor("w1", [128, H], WDT, kind="ExternalInput")
    w2 = nc.dram_tensor("w2", [128, KBLK, H], WDT, kind="ExternalInput")
    w3 = nc.dram_tensor("w3", [128, KBLK, H], WDT, kind="ExternalInput")
    w4 = nc.dram_tensor("w4", [128, KBLK, H], WDT, kind="ExternalInput")
    wo = nc.dram_tensor("wo", [128, KBLK, OUTW], WDT, kind="ExternalInput")
    bh = nc.dram_tensor("bh", [128, 16], F32, kind="ExternalInput")
    bo = nc.dram_tensor("bo", [64, 1], F32, kind="ExternalInput")
    lds = nc.dram_tensor("lds", [64, 2], F32, kind="ExternalInput")

    om = nc.dram_tensor("om", [B, OBS], F32, kind="ExternalOutput")
    os_ = nc.dram_tensor("os", [B, OBS], F32, kind="ExternalOutput")
    ord_ = nc.dram_tensor("ord", [2, B], F32, kind="ExternalOutput")

    with tile.TileContext(nc) as tc:
        with (
            tc.tile_pool(name="wp", bufs=1) as wp,
            tc.tile_pool(name="a0p", bufs=6) as a0p,
            tc.tile_pool(name="ap", bufs=20) as ap,
            tc.tile_pool(name="op", bufs=3) as op,
            tc.tile_pool(name="php", bufs=5, space="PSUM") as php,
            tc.tile_pool(name="pop", bufs=3, space="PSUM") as pop,
        ):
            w1_sb = wp.tile([128, H], WDT, tag="w1")
            w2_sb = wp.tile([128, KBLK, H], WDT, tag="w2")
            w3_sb = wp.tile([128, KBLK, H], WDT, tag="w3")
            w4_sb = wp.tile([128, KBLK, H], WDT, tag="w4")
            wo_sb = wp.tile([128, KBLK, OUTW], WDT, tag="wo")
            bh_sb = wp.tile([128, 16], F32, tag="bh")
            bo_sb = wp.tile([64, 1], F32, tag="bo")
            lds_sb = wp.tile([64, 2], F32, tag="lds")
            nc.sync.dma_start(w1_sb[:], w1[:])
            nc.sync.dma_start(w2_sb[:], w2[:])
            nc.sync.dma_start(w3_sb[:], w3[:])
            nc.sync.dma_start(w4_sb[:], w4[:])
            nc.sync.dma_start(wo_sb[:], wo[:])
            nc.sync.dma_start(bh_sb[:], bh[:])
            nc.sync.dma_start(bo_sb[:], bo[:])
            nc.sync.dma_start(lds_sb[:], lds[:])

            w_mid = [w2_sb, w3_sb, w4_sb]

            def relu_into(dst, src, bias_ap, use_act):
                """dst = relu(src + bias), on ACT or DVE."""
                if use_act:
                    nc.scalar.activation(dst, src, AF.Relu, bias=bias_ap)
                else:
                    nc.vector.tensor_scalar(
                        out=dst, in0=src, scalar1=bias_ap, scalar2=0.0,
                        op0=ALU.add, op1=ALU.max,
                    )

            def load_a0(i):
                a0 = a0p.tile([128, CHUNK], ADT, tag="a0", name=f"a0_{i}")
                nc.sync.dma_start(a0[:], xt[:, bass.ts(i, CHUNK)])
                return a0

            def layer1(i, a0):
                a_out = []
                for c in range(KBLK):
                    ph = php.tile([128, CHUNK], F32, tag="ph", name=f"ph1_{i}_{c}")
                    nc.tensor.matmul(
                        ph[:], w1_sb[:, bass.ts(c, 128)], a0[:],
                        start=True, stop=True,
                    )
                    a = ap.tile([128, CHUNK], ADT, tag="a", name=f"a1_{i}_{c}")
                    relu_into(a[:], ph[:], bh_sb[:, c : c + 1], use_act=(c < 2))
                    a_out.append(a)
                return a_out

            def layer_mid(i, li, a_in):
                w_sb = w_mid[li]
                a_out = []
                for c in range(KBLK):
                    ph = php.tile([128, CHUNK], F32, tag="ph", name=f"ph_{i}_{li}_{c}")
                    for k in range(KBLK):
                        nc.tensor.matmul(
                            ph[:], w_sb[:, k, bass.ts(c, 128)], a_in[k][:],
                            start=(k == 0), stop=(k == KBLK - 1),
                        )
                    a = ap.tile([128, CHUNK], ADT, tag="a", name=f"a_{i}_{li}_{c}")
                    bias_ap = bh_sb[:, 4 * (li + 1) + c : 4 * (li + 1) + c + 1]
                    relu_into(a[:], ph[:], bias_ap, use_act=(c < 2))
                    a_out.append(a)
                return a_out

            def out_head(i, a_in):
                po = pop.tile([OUTW, CHUNK], F32, tag="po", name=f"po_{i}")
                for k in range(KBLK):
                    nc.tensor.matmul(
                        po[:], wo_sb[:, k, :], a_in[k][:],
                        start=(k == 0), stop=(k == KBLK - 1),
                    )
                return po

            # Post-processing, split into stages that are emitted a full layer
            # (~3.4us) apart so no engine ever dequeues an op whose cross-engine
            # dependency is still pending (single counting sem per engine ->
            # any wait at the queue head stalls everything behind it).
            #
            # The logstd clip+exp runs entirely on ACT via relu identities:
            #   z  = max(x + bls, -10)  = relu(x + (bls+10)) - 10
            #   w  = min(z, 0.5)        = 0.5 - relu(10.5 - relu(x + bls + 10))
            #   std = exp(w + log(dstd)) = exp(-r2 + (0.5 + log(dstd)))
            # bo rows 32:64 hold bls+10; lds rows 32:64 hold 0.5+log(dstd).
            def post_stage1(i, po):
                t_out = op.tile([64, CHUNK], F32, tag="t_out", name=f"to_{i}")
                t_r1 = op.tile([64, CHUNK], F32, tag="t_r1", name=f"t1_{i}")
                # means: + bias (denorm folded into weights) — DVE
                nc.vector.tensor_scalar_add(
                    out=t_out[0:32, :], in0=po[0:32, :], scalar1=bo_sb[0:32, :],
                )
                # r1 = relu(x + bls + 10) — ACT
                nc.scalar.activation(
                    t_r1[32:64, :], po[32:64, :], AF.Relu, bias=bo_sb[32:64, :],
                )
                # rewards/dones raw (bias added on host; DMA can't read PSUM)
                t_rd = op.tile([66, CHUNK], F32, tag="t_rd", name=f"tr_{i}")
                nc.scalar.copy(t_rd[64:66, :], po[64:66, :])
                return t_out, t_r1, t_rd

            def post_stage2(i, st):
                t_out, t_r1, t_rd = st
                t_r2 = op.tile([64, CHUNK], F32, tag="t_r2", name=f"t2_{i}")
                # r2 = relu(10.5 - r1) — ACT
                nc.scalar.activation(
                    t_r2[32:64, :], t_r1[32:64, :], AF.Relu,
                    bias=lds_sb[32:64, 1:2], scale=-1.0,
                )
                return t_out, t_r2, t_rd

            def post_stage3(i, st):
                t_out, t_r2, t_rd = st
                # std = exp(-r2 + (0.5 + log(dstd))) — ACT
                nc.scalar.activation(
                    t_out[32:64, :], t_r2[32:64, :], AF.Exp,
                    bias=lds_sb[32:64, 0:1], scale=-1.0,
                )
                return t_out, t_rd

            def post_stage4(i, st):
                t_out, t_rd = st
                cs = bass.ts(i, CHUNK)
                # block-transpose [feat, batch] -> batch-major and store
                t_tr = op.tile([64, CHUNK], F32, tag="t_tr", name=f"tt_{i}")
                nc.vector.transpose(t_tr[:], t_out[:])
                nc.sync.dma_start(
                    om[cs, :].rearrange("(j p) q -> p j q", p=32),
                    t_tr[0:32, :].rearrange("p (j q) -> p j q", q=32),
                )
                nc.sync.dma_start(
                    os_[cs, :].rearrange("(j p) q -> p j q", p=32),
                    t_tr[32:64, :].rearrange("p (j q) -> p j q", q=32),
                )
                nc.sync.dma_start(ord_[:, cs], t_rd[64:66, :])

            # Chunk PAIRS, layer-interleaved: layer l of chunk B issues between
            # layer l and l+1 of chunk A, so every matmul's A-tile inputs were
            # produced >= one full layer (16 matmuls, ~3.4us) earlier — relu
            # latency never stalls the PE at layer transitions.
            #
            # Post-processing of pair p is DEFERRED until after pair p+1's
            # matmul/relu stream: each engine has one counting semaphore, so a
            # slow post chain queued between relu streams would head-of-line
            # block the semaphore ticks the next pair's matmuls wait on. By
            # post time all inputs are a full pair (~13us) old, so the post
            # ops stream through ACT/DVE without wait-stalls.
            def run_post_all(pend):
                for i, st in pend:
                    st = post_stage2(i, post_stage1(i, st))
                    post_stage4(i, post_stage3(i, st))

            pending = None
            for p in range(N_CHUNKS // 2):
                iA, iB = 2 * p, 2 * p + 1
                a0A, a0B = load_a0(iA), load_a0(iB)
                aA, aB = layer1(iA, a0A), layer1(iB, a0B)
                if pending is not None:
                    st = [post_stage1(i, po) for i, po in pending]
                aA, aB = layer_mid(iA, 0, aA), layer_mid(iB, 0, aB)
                if pending is not None:
                    st = [post_stage2(pending[j][0], s) for j, s in enumerate(st)]
                aA, aB = layer_mid(iA, 1, aA), layer_mid(iB, 1, aB)
                if pending is not None:
                    st = [post_stage3(pending[j][0], s) for j, s in enumerate(st)]
                aA, aB = layer_mid(iA, 2, aA), layer_mid(iB, 2, aB)
                if pending is not None:
                    for j, s in enumerate(st):
                        post_stage4(pending[j][0], s)
                poA = out_head(iA, aA)
                poB = out_head(iB, aB)
                pending = ((iA, poA), (iB, poB))
            run_post_all(pending)

    nc.compile()
    return nc


def _prep_core_inputs(xt_full, W1, b1, W2, b2, W3, b3, W4, b4,
                      Wm, bm, Wls, bls, Wr, br, Wd, bd,
                      state_mean, state_std, action_mean, action_std,
                      delta_mean, delta_std, e):
    f64 = np.float64
    mu = np.concatenate([state_mean, action_mean]).astype(f64)
    sig = np.concatenate([state_std, action_std]).astype(f64)

    W1e = W1[e].astype(f64)
    w1f = W1e / sig[:, None]
    b1f = b1[e].astype(f64) - (mu / sig) @ W1e

    wnp = np.float32
    if WEIGHT_BF16:
        import ml_dtypes
        wnp = ml_dtypes.bfloat16

    def blocks(w, width):
        return np.ascontiguousarray(
            w.reshape(KBLK, 128, width).transpose(1, 0, 2)
        ).astype(wnp)

    dstd = delta_std.astype(f64)
    dmean = delta_mean.astype(f64)
    wm_f = Wm[e].astype(f64) * dstd[None, :]
    # rows 0:32: means bias (denorm folded); rows 32:64: bls + 10 (for the
    # relu-based lower clip on ACT)
    bo_v = np.concatenate(
        [bm[e].astype(f64) * dstd + dmean, bls[e].astype(f64) + 10.0]
    ).astype(np.float32)
    wo_full = np.concatenate(
        [wm_f.astype(np.float32), Wls[e], Wr[e], Wd[e]], axis=1
    )  # [H, 66]

    bh_v = np.zeros((128, 16), np.float32)
    for l, bl in enumerate([b1f.astype(np.float32), b2[e], b3[e], b4[e]]):
        for c in range(KBLK):
            bh_v[:, 4 * l + c] = bl[c * 128 : (c + 1) * 128]

    lds_v = np.zeros((64, 2), np.float32)
    lds_v[32:64, 0] = (0.5 + np.log(dstd)).astype(np.float32)
    lds_v[32:64, 1] = 10.5

    w1p = np.zeros((128, H), wnp)
    w1p[:D_IN, :] = w1f.astype(wnp)

    return {
        "xt": xt_full,
        "w1": w1p,
        "w2": blocks(W2[e], H),
        "w3": blocks(W3[e], H),
        "w4": blocks(W4[e], H),
        "wo": blocks(wo_full, OUTW),
        "bh": bh_v,
        "bo": bo_v[:, None].copy(),
        "lds": lds_v,
    }


def kernel(states, actions, state_mean, state_std, action_mean, action_std,
           delta_mean, delta_std,
           W1, b1, W2, b2, W3, b3, W4, b4,
           Wm, bm, Wls, bls, Wr, br, Wd, bd, **run_kwargs):
    from concourse.bass_utils import run_bass_kernel_spmd

    to_np = lambda a: np.asarray(a, dtype=np.float32)
    states, actions = to_np(states), to_np(actions)
    args = [to_np(a) for a in (W1, b1, W2, b2, W3, b3, W4, b4,
                               Wm, bm, Wls, bls, Wr, br, Wd, bd)]
    (W1, b1, W2, b2, W3, b3, W4, b4,
     Wm, bm, Wls, bls, Wr, br, Wd, bd) = args
    norms = [to_np(a) for a in (state_mean, state_std, action_mean, action_std,
                                delta_mean, delta_std)]
    (state_mean, state_std, action_mean, action_std,
     delta_mean, delta_std) = norms

    if "nc" not in _cache:
        _cache["nc"] = _build_program()
    nc = _cache["nc"]

    xnp = np.float32
    if WEIGHT_BF16:
        import ml_dtypes
        xnp = ml_dtypes.bfloat16
    xt_full = np.zeros((128, B), xnp)
    xt_full[:D_IN, :] = np.concatenate([states, actions], axis=1).T.astype(xnp)

    in_maps = [
        _prep_core_inputs(xt_full, W1, b1, W2, b2, W3, b3, W4, b4,
                          Wm, bm, Wls, bls, Wr, br, Wd, bd,
                          state_mean, state_std, action_mean, action_std,
                          delta_mean, delta_std, e)
        for e in range(N_CORES)
    ]

    res = run_bass_kernel_spmd(nc, in_maps, list(range(N_CORES)), **run_kwargs)
    _cache["last_result"] = res

    means = np.stack([res.results[e]["om"] for e in range(N_CORES)])
    stds = np.stack([res.results[e]["os"] for e in range(N_CORES)])
    rd = np.stack([res.results[e]["ord"] for e in range(N_CORES)])  # [E, 2, B]
    rewards = (rd[:, 0, :] + br[:, 0:1]).astype(np.float32)[:, :, None]
    dones = (rd[:, 1, :] + bd[:, 0:1]).astype(np.float32)[:, :, None]
    return means, stds, rewards, dones
